# revision 8
# baseline (speedup 1.0000x reference)
"""AttentionPooling (ragged segment attention) on 8 Trainium2 NeuronCores.

Full inputs in, full output out. Strategy (data-parallel over graphs):
  - 128 graphs are load-balanced 16-per-core across 8 cores; each core gets
    its graphs' node embeddings (zero-padded to a multiple of 512 rows).
  - The single shared query is a model parameter, so the q-side is constant-
    folded on the host:  qk[h,e] = sum_d q_scaled[h,d]*k_w[h*64+d,e].
  - On device (per core), with cols c = h*16 + s (8 heads x 16 graph slots):
      scoresT[c, n] = sum_e qk_cols[e,c] * embT[e,n]          (PE, bf16)
      e[c, n]       = exp(scoresT + qb[c]) * indicator[c, n]  (ACT + DVE)
      e_cols        = PE-transpose(e)                          [n, c]
      pooled[c, :]  = sum_n e_cols[n,c] * emb[n,:]            (PE, accum)
      colsum[c]     = sum_n e_cols[n,c]  (+ host phantom correction)
      pooled       /= colsum                                  (DVE)
      o[s-block]    = blockdiag v-proj, then out-proj          (PE)
  - Host gathers the 8x[16,512] results back to [bs, 512].
"""

import numpy as np
import ml_dtypes

BF16 = ml_dtypes.bfloat16
E = 768
D = 512
H = 8
DH = 64
NCORES = 8
SLOTS = 16          # graphs per core
COLS = 128          # H * SLOTS
ES = E // 128       # 6 E-slices of 128

_prog_cache = {}


def _build_program(nc_pad):
    import concourse.bacc as bacc
    import concourse.tile as tile
    import concourse.mybir as mybir

    f32 = mybir.dt.float32
    bf16 = mybir.dt.bfloat16
    AF = mybir.ActivationFunctionType

    nc = bacc.Bacc(None, target_bir_lowering=False)

    emb_d = nc.declare_dram_parameter("emb", [nc_pad, E], bf16, isOutput=False)
    embT_d = nc.declare_dram_parameter("embT", [E, nc_pad], bf16, isOutput=False)
    ind_d = nc.declare_dram_parameter("indT", [COLS, nc_pad], bf16, isOutput=False)
    qk_d = nc.declare_dram_parameter("qk", [E, COLS], bf16, isOutput=False)
    ph_d = nc.declare_dram_parameter("ph", [COLS, 1], f32, isOutput=False)
    vT_d = nc.declare_dram_parameter("vT", [E, D], bf16, isOutput=False)
    owT_d = nc.declare_dram_parameter("owT", [D, D], bf16, isOutput=False)
    ob_d = nc.declare_dram_parameter("ob", [SLOTS, D], f32, isOutput=False)
    id_d = nc.declare_dram_parameter("ident", [128, 128], bf16, isOutput=False)
    ones_d = nc.declare_dram_parameter("ones", [128, 1], bf16, isOutput=False)
    out_d = nc.declare_dram_parameter("out", [SLOTS, D], f32, isOutput=True)

    NGRP = nc_pad // 512         # 512-node groups
    NCH = nc_pad // 128          # 128-node chunks

    with tile.TileContext(nc) as tc:
        with (
            tc.tile_pool(name="const", bufs=1) as const,
            tc.tile_pool(name="embT_p", bufs=2) as embT_p,
            tc.tile_pool(name="emb_p", bufs=4) as emb_p,
            tc.tile_pool(name="e_p", bufs=3) as e_p,
            tc.tile_pool(name="ec_p", bufs=4) as ec_p,
            tc.tile_pool(name="small", bufs=1) as small,
            tc.tile_pool(name="psc", bufs=2, space="PSUM") as psc,
            tc.tile_pool(name="pst", bufs=2, space="PSUM") as pst,
            tc.tile_pool(name="pacc", bufs=1, space="PSUM") as pacc,
        ):
            # ---- constants into SBUF ----
            qk_sb = const.tile([128, ES, COLS], bf16)
            nc.sync.dma_start(out=qk_sb, in_=qk_d.rearrange("(s p) c -> p s c", p=128))
            ph_sb = const.tile([COLS, 1], f32)
            nc.sync.dma_start(out=ph_sb, in_=ph_d[:, :])
            ind_sb = const.tile([COLS, nc_pad], bf16)
            nc.sync.dma_start(out=ind_sb, in_=ind_d[:, :])
            vT_sb = const.tile([128, ES, D], bf16)
            nc.sync.dma_start(out=vT_sb, in_=vT_d.rearrange("(s p) c -> p s c", p=128))
            owT_sb = const.tile([128, 4, D], bf16)
            nc.sync.dma_start(out=owT_sb, in_=owT_d.rearrange("(s p) c -> p s c", p=128))
            ob_sb = const.tile([SLOTS, D], f32)
            nc.sync.dma_start(out=ob_sb, in_=ob_d[:, :])
            id_sb = const.tile([128, 128], bf16)
            nc.sync.dma_start(out=id_sb, in_=id_d[:, :])
            ones_sb = const.tile([128, 1], bf16)
            nc.sync.dma_start(out=ones_sb, in_=ones_d[:, :])

            # ---- persistent accumulators (PSUM) ----
            ps_pool = pacc.tile([COLS, E], f32)      # pooled_u, 2 banks
            ps_cs = pacc.tile([COLS, 1], f32)        # col sums, 1 bank

            embT_r = embT_d.rearrange("(s p) n -> p s n", p=128)

            for g in range(NGRP):
                # load embT group as 6 per-slice tiles [128, 512]
                ets = []
                for s in range(ES):
                    t = embT_p.tile([128, 512], bf16, tag=f"et{s}")
                    nc.sync.dma_start(out=t, in_=embT_r[:, s, g * 512:(g + 1) * 512])
                    ets.append(t)

                # scoresT[c, n] accumulate over 6 E-slices
                ps_s = psc.tile([COLS, 512], f32, tag="s")
                for s in range(ES):
                    nc.tensor.matmul(
                        ps_s, lhsT=qk_sb[:, s, :], rhs=ets[s],
                        start=(s == 0), stop=(s == ES - 1),
                    )

                # e = exp(scores + qb); mask by indicator
                e_sb = e_p.tile([COLS, 512], bf16, tag="e")
                nc.scalar.activation(out=e_sb, in_=ps_s, func=AF.Exp)
                em_sb = e_p.tile([COLS, 512], bf16, tag="em")
                nc.vector.tensor_mul(em_sb, e_sb, ind_sb[:, g * 512:(g + 1) * 512])

                for j in range(4):
                    ch = g * 4 + j
                    # e_cols chunk: [128 nodes, 128 cols] via PE transpose
                    ps_t = pst.tile([128, 128], bf16, tag="tr")
                    nc.tensor.transpose(ps_t, em_sb[:, j * 128:(j + 1) * 128], id_sb)
                    ec = ec_p.tile([128, COLS], bf16)
                    nc.vector.tensor_copy(ec, ps_t)

                    # natural-layout emb chunk
                    embt = emb_p.tile([128, E], bf16)
                    nc.sync.dma_start(out=embt, in_=emb_d[ch * 128:(ch + 1) * 128, :])

                    st = (ch == 0)
                    sp = (ch == NCH - 1)
                    nc.tensor.matmul(ps_pool[:, 0:512], lhsT=ec, rhs=embt[:, 0:512],
                                     start=st, stop=sp)
                    nc.tensor.matmul(ps_pool[:, 512:768], lhsT=ec, rhs=embt[:, 512:768],
                                     start=st, stop=sp)
                    nc.tensor.matmul(ps_cs, lhsT=ec, rhs=ones_sb, start=st, stop=sp)

            # ---- normalize ----
            cs_sb = small.tile([COLS, 1], f32)
            nc.vector.tensor_add(cs_sb, ps_cs, ph_sb)
            rec_sb = small.tile([COLS, 1], f32)
            nc.vector.reciprocal(rec_sb, cs_sb)
            pooled = small.tile([COLS, E], bf16)
            nc.vector.tensor_scalar_mul(pooled, in0=ps_pool, scalar1=rec_sb)

            # ---- pooledT via PE transposes ----
            pT = small.tile([128, ES, COLS], bf16)
            for s in range(ES):
                ps_t2 = pst.tile([128, 128], bf16, tag="tr")
                nc.tensor.transpose(ps_t2, pooled[:, s * 128:(s + 1) * 128], id_sb)
                nc.vector.tensor_copy(pT[:, s, :], ps_t2)

            # ---- v-projection: o_full[c, j] = sum_e pooled[c, e] * v_w[j, e] ----
            ps_o = psc.tile([COLS, D], f32, tag="s")
            for s in range(ES):
                nc.tensor.matmul(ps_o, lhsT=pT[:, s, :], rhs=vT_sb[:, s, :],
                                 start=(s == 0), stop=(s == ES - 1))

            # ---- diag extract: oS[g, h*64:(h+1)*64] = ps_o[h*16+g, h*64:(h+1)*64]
            o_sb = small.tile([COLS, D], bf16)
            nc.vector.tensor_copy(o_sb, ps_o)
            oS = small.tile([SLOTS, D], bf16)
            for h in range(H):
                nc.sync.dma_start(
                    out=oS[:, h * DH:(h + 1) * DH],
                    in_=o_sb[h * SLOTS:(h + 1) * SLOTS, h * DH:(h + 1) * DH],
                )

            # ---- oT via PE transposes: [16, 512] -> 4 x [128, 16] ----
            oT = small.tile([128, 4, SLOTS], bf16)
            for s in range(4):
                ps_t3 = pst.tile([128, 128], bf16, tag="tr")
                nc.tensor.transpose(ps_t3[:, 0:SLOTS], oS[:, s * 128:(s + 1) * 128],
                                    id_sb[0:SLOTS, 0:SLOTS])
                nc.vector.tensor_copy(oT[:, s, :], ps_t3[:, 0:SLOTS])

            # ---- out-projection: out[g, j] = sum_i o[g, i] * out_w[j, i] ----
            ps_f = psc.tile([SLOTS, D], f32, tag="s")
            for s in range(4):
                nc.tensor.matmul(ps_f, lhsT=oT[:, s, :], rhs=owT_sb[:, s, :],
                                 start=(s == 0), stop=(s == 3))

            res = small.tile([SLOTS, D], f32)
            nc.vector.tensor_add(res, ps_f, ob_sb)
            nc.sync.dma_start(out=out_d[:, :], in_=res)

    nc.finalize()
    return nc


def _host_prep(graph_emb, qry, q_w, k_w, v_w, in_b, out_w, out_b, ptr, batch):
    graph_emb = np.asarray(graph_emb, dtype=np.float32)
    qry = np.asarray(qry, dtype=np.float32)
    q_w = np.asarray(q_w, dtype=np.float32)
    k_w = np.asarray(k_w, dtype=np.float32)
    v_w = np.asarray(v_w, dtype=np.float32)
    in_b = np.asarray(in_b, dtype=np.float32)
    out_w = np.asarray(out_w, dtype=np.float32)
    out_b = np.asarray(out_b, dtype=np.float32)
    ptr = np.asarray(ptr).astype(np.int64)
    batch = np.asarray(batch).astype(np.int64)

    N = graph_emb.shape[0]
    B = len(ptr) - 1
    n_nodes = ptr[1:] - ptr[:-1]
    max_node = int(n_nodes.max()) + 1
    bs = int(batch.max()) + 1

    # --- mirror the reference's scatter semantics (jnp .at[] wraps negatives,
    # drops OOB, last write wins; valid mask is by slot index) ---
    pos = np.arange(N) - ptr[batch]
    m = np.where(pos < 0, pos + max_node, pos)
    part = (m >= 0) & (m < max_node) & (m < n_nodes[batch])
    idx = np.nonzero(part)[0]
    key = batch[idx] * max_node + m[idx]
    _, first_rev = np.unique(key[::-1], return_index=True)
    keep = idx[::-1][first_rev]
    keep.sort()
    kb = batch[keep]
    counts = np.bincount(kb, minlength=B)
    phantom = n_nodes.astype(np.float64) - counts  # valid-but-unfilled slots

    # --- q-side constant folding (qry is a model parameter) ---
    bq, bk, bv = in_b[:D], in_b[D:2 * D], in_b[2 * D:]
    scale = DH ** -0.5
    q = ((qry.reshape(-1)[-D:] @ q_w.T) + bq) * scale
    qh = q.reshape(H, DH)
    qk = np.stack([qh[h] @ k_w[h * DH:(h + 1) * DH, :] for h in range(H)])  # [8, E]
    qb = np.einsum("hd,hd->h", qh, bk.reshape(H, DH))                        # [8]
    ob_eff = out_b + out_w @ bv

    # --- balanced assignment: 16 graphs per core, boustrophedon by size ---
    order = np.argsort(-counts, kind="stable")
    slot_of = np.empty(B, dtype=np.int64)   # graph -> core*16+slot
    for r in range(SLOTS):
        row = order[r * NCORES:(r + 1) * NCORES]
        seq = range(NCORES) if r % 2 == 0 else range(NCORES - 1, -1, -1)
        for c, gi in zip(seq, row):
            slot_of[gi] = c * SLOTS + r

    nodes_of = [[] for _ in range(B)]
    for n in keep:
        nodes_of[batch[n]].append(n)

    core_loads = np.zeros(NCORES, dtype=np.int64)
    for gi in range(B):
        core_loads[slot_of[gi] // SLOTS] += counts[gi]
    nc_pad = max(512, int(np.ceil(core_loads.max() / 512.0)) * 512)

    exp_qb = np.exp(qb)

    in_maps = []
    for c in range(NCORES):
        rows = []
        ind16 = np.zeros((SLOTS, nc_pad), dtype=BF16)
        ph_col = np.zeros((COLS, 1), dtype=np.float32)
        off = 0
        for s in range(SLOTS):
            gis = np.nonzero(slot_of == c * SLOTS + s)[0]
            if len(gis) == 0:
                continue
            gi = int(gis[0])
            ns = nodes_of[gi]
            rows.extend(ns)
            ind16[s, off:off + len(ns)] = 1
            off += len(ns)
            for h in range(H):
                ph_col[h * SLOTS + s, 0] = phantom[gi]
        emb_c = np.zeros((nc_pad, E), dtype=BF16)
        if rows:
            emb_c[:len(rows)] = graph_emb[np.asarray(rows)].astype(BF16)
        in_maps.append({
            "emb": emb_c,
            "embT": np.ascontiguousarray(emb_c.T),
            "indT": np.ascontiguousarray(np.tile(ind16, (H, 1))),
            "qk": np.ascontiguousarray(np.repeat(qk, SLOTS, axis=0).T.astype(BF16)),
            "ph": ph_col,
            "vT": np.ascontiguousarray(v_w.T).astype(BF16),
            "owT": np.ascontiguousarray(out_w.T).astype(BF16),
            "ob": np.broadcast_to(ob_eff, (SLOTS, D)).astype(np.float32).copy(),
            "ident": np.eye(128, dtype=BF16),
            "ones": np.ones((128, 1), dtype=BF16),
        })

    meta = {
        "bs": bs,
        "slot_of": slot_of,
        "n_nodes": n_nodes,
        "nc_pad": nc_pad,
    }
    return in_maps, meta


def _assemble(results, meta):
    bs = meta["bs"]
    slot_of = meta["slot_of"]
    n_nodes = meta["n_nodes"]
    out = np.empty((bs, D), dtype=np.float32)
    for b in range(bs):
        sl = int(slot_of[b])
        out[b] = results[sl // SLOTS]["out"][sl % SLOTS]
        if n_nodes[b] <= 0:
            out[b] = np.nan
    return out


def kernel(graph_emb, qry, q_w, k_w, v_w, in_b, out_w, out_b, ptr, batch):
    from concourse.bass_utils import run_bass_kernel_spmd

    in_maps, meta = _host_prep(graph_emb, qry, q_w, k_w, v_w, in_b, out_w,
                               out_b, ptr, batch)
    nc_pad = meta["nc_pad"]
    if nc_pad not in _prog_cache:
        _prog_cache[nc_pad] = _build_program(nc_pad)
    nc = _prog_cache[nc_pad]
    res = run_bass_kernel_spmd(nc, in_maps, list(range(NCORES)))
    return _assemble(res.results, meta)


# revision 10
# speedup vs baseline: 1.1097x; 1.1097x over previous
"""AttentionPooling (ragged segment attention) on 8 Trainium2 NeuronCores.

Full inputs in, full output out. Strategy (data-parallel over graphs):
  - 128 graphs are load-balanced 16-per-core across 8 cores; each core gets
    its graphs' node embeddings (zero-padded to a multiple of 512 rows).
  - The single shared query is a model parameter, so the q-side is constant-
    folded on the host:  qk[h,e] = sum_d q_scaled[h,d]*k_w[h*64+d,e].
  - On device (per core), with cols c = h*16 + s (8 heads x 16 graph slots):
      scoresT[c, n] = sum_e qk_cols[e,c] * embT[e,n]          (PE, bf16)
      e[c, n]       = exp(scoresT + qb[c]) * indicator[c, n]  (ACT + DVE)
      e_cols        = PE-transpose(e)                          [n, c]
      pooled[c, :]  = sum_n e_cols[n,c] * emb[n,:]            (PE, accum)
      colsum[c]     = sum_n e_cols[n,c]  (+ host phantom correction)
      pooled       /= colsum                                  (DVE)
      o[s-block]    = blockdiag v-proj, then out-proj          (PE)
  - Host gathers the 8x[16,512] results back to [bs, 512].
"""

import numpy as np
import ml_dtypes

BF16 = ml_dtypes.bfloat16
E = 768
D = 512
H = 8
DH = 64
NCORES = 8
SLOTS = 16          # graphs per core
COLS = 128          # H * SLOTS
ES = E // 128       # 6 E-slices of 128

_prog_cache = {}


def _build_program(nc_pad):
    import concourse.bacc as bacc
    import concourse.tile as tile
    import concourse.mybir as mybir

    f32 = mybir.dt.float32
    bf16 = mybir.dt.bfloat16
    AF = mybir.ActivationFunctionType

    nc = bacc.Bacc(None, target_bir_lowering=False)

    emb_d = nc.declare_dram_parameter("emb", [nc_pad, E], bf16, isOutput=False)
    NGRP_ = nc_pad // 512
    embT_d = nc.declare_dram_parameter("embT", [NGRP_ * 128, ES * 512], bf16, isOutput=False)
    ind_d = nc.declare_dram_parameter("indT", [COLS, nc_pad], bf16, isOutput=False)
    qk_d = nc.declare_dram_parameter("qk", [E, COLS], bf16, isOutput=False)
    ph_d = nc.declare_dram_parameter("ph", [COLS, 1], f32, isOutput=False)
    vT_d = nc.declare_dram_parameter("vT", [E, D], bf16, isOutput=False)
    owT_d = nc.declare_dram_parameter("owT", [D, D], bf16, isOutput=False)
    ob_d = nc.declare_dram_parameter("ob", [SLOTS, D], f32, isOutput=False)
    id_d = nc.declare_dram_parameter("ident", [128, 128], bf16, isOutput=False)
    ones_d = nc.declare_dram_parameter("ones", [128, 1], bf16, isOutput=False)
    out_d = nc.declare_dram_parameter("out", [SLOTS, D], f32, isOutput=True)

    NGRP = nc_pad // 512         # 512-node groups
    NCH = nc_pad // 128          # 128-node chunks

    with tile.TileContext(nc) as tc:
        with (
            tc.tile_pool(name="const", bufs=1) as const,
            tc.tile_pool(name="embT_p", bufs=3) as embT_p,
            tc.tile_pool(name="emb_p", bufs=10) as emb_p,
            tc.tile_pool(name="e_p", bufs=3) as e_p,
            tc.tile_pool(name="ec_p", bufs=6) as ec_p,
            tc.tile_pool(name="small", bufs=1) as small,
            tc.tile_pool(name="psc", bufs=2, space="PSUM") as psc,
            tc.tile_pool(name="pst", bufs=2, space="PSUM") as pst,
            tc.tile_pool(name="pacc", bufs=1, space="PSUM") as pacc,
        ):
            # ---- constants into SBUF ----
            qk_sb = const.tile([128, ES, COLS], bf16)
            nc.sync.dma_start(out=qk_sb, in_=qk_d.rearrange("(s p) c -> p s c", p=128))
            ph_sb = const.tile([COLS, 1], f32)
            nc.sync.dma_start(out=ph_sb, in_=ph_d[:, :])
            ind_sb = const.tile([COLS, nc_pad], bf16)
            nc.sync.dma_start(out=ind_sb, in_=ind_d[:, :])
            vT_sb = const.tile([128, ES, D], bf16)
            nc.sync.dma_start(out=vT_sb, in_=vT_d.rearrange("(s p) c -> p s c", p=128))
            owT_sb = const.tile([128, 4, D], bf16)
            nc.sync.dma_start(out=owT_sb, in_=owT_d.rearrange("(s p) c -> p s c", p=128))
            ob_sb = const.tile([SLOTS, D], f32)
            nc.sync.dma_start(out=ob_sb, in_=ob_d[:, :])
            id_sb = const.tile([128, 128], bf16)
            nc.sync.dma_start(out=id_sb, in_=id_d[:, :])
            ones_sb = const.tile([128, 1], bf16)
            nc.sync.dma_start(out=ones_sb, in_=ones_d[:, :])

            # ---- persistent accumulators (PSUM) ----
            ps_pool = pacc.tile([COLS, E], f32)      # pooled_u, 2 banks
            ps_cs = pacc.tile([COLS, 1], f32)        # col sums, 1 bank

            # embT is shipped pre-grouped: [NGRP, 128, ES, 512] with each
            # group's block contiguous in DRAM for clean big DMAs.
            embT_r = embT_d.rearrange("(g p) (s n) -> g p s n", p=128, n=512)

            def emit_scores(g):
                et = embT_p.tile([128, ES, 512], bf16, tag="et")
                nc.sync.dma_start(out=et, in_=embT_r[g])
                ps_s = psc.tile([COLS, 512], f32, tag="s")
                for s in range(ES):
                    nc.tensor.matmul(
                        ps_s, lhsT=qk_sb[:, s, :], rhs=et[:, s, :],
                        start=(s == 0), stop=(s == ES - 1),
                    )
                # e = exp(scores); mask by indicator (qb folded out, see host)
                e_sb = e_p.tile([COLS, 512], bf16, tag="e")
                nc.scalar.activation(out=e_sb, in_=ps_s, func=AF.Exp)
                em_sb = e_p.tile([COLS, 512], bf16, tag="em")
                nc.vector.tensor_mul(em_sb, e_sb, ind_sb[:, g * 512:(g + 1) * 512])
                embts = []
                for j in range(4):
                    ch = g * 4 + j
                    embt = emb_p.tile([128, E], bf16)
                    nc.sync.dma_start(out=embt, in_=emb_d[ch * 128:(ch + 1) * 128, :])
                    embts.append(embt)
                return em_sb, embts

            def emit_pool(g, em_sb, embts):
                for j in range(4):
                    ch = g * 4 + j
                    # e_cols chunk: [128 nodes, 128 cols] via PE transpose
                    ps_t = pst.tile([128, 128], bf16, tag="tr")
                    nc.tensor.transpose(ps_t, em_sb[:, j * 128:(j + 1) * 128], id_sb)
                    ec = ec_p.tile([128, COLS], bf16)
                    nc.vector.tensor_copy(ec, ps_t)

                    embt = embts[j]
                    st = (ch == 0)
                    sp = (ch == NCH - 1)
                    nc.tensor.matmul(ps_pool[:, 0:512], lhsT=ec, rhs=embt[:, 0:512],
                                     start=st, stop=sp)
                    nc.tensor.matmul(ps_pool[:, 512:768], lhsT=ec, rhs=embt[:, 512:768],
                                     start=st, stop=sp)
                    nc.tensor.matmul(ps_cs, lhsT=ec, rhs=ones_sb, start=st, stop=sp)

            # Software pipeline: PE stays busy on group g+1's scores while
            # ACT/DVE produce group g's masked-exp, whose transposes+pools
            # are emitted (in PE program order) after those scores.
            prev = None
            for g in range(NGRP):
                em, embts = emit_scores(g)
                if prev is not None:
                    emit_pool(*prev)
                prev = (g, em, embts)
            emit_pool(*prev)

            # ---- normalize ----
            cs_sb = small.tile([COLS, 1], f32)
            nc.vector.tensor_add(cs_sb, ps_cs, ph_sb)
            rec_sb = small.tile([COLS, 1], f32)
            nc.vector.reciprocal(rec_sb, cs_sb)
            pooled = small.tile([COLS, E], bf16)
            nc.vector.tensor_scalar_mul(pooled, in0=ps_pool, scalar1=rec_sb)

            # ---- pooledT via PE transposes ----
            pT = small.tile([128, ES, COLS], bf16)
            for s in range(ES):
                ps_t2 = pst.tile([128, 128], bf16, tag="tr")
                nc.tensor.transpose(ps_t2, pooled[:, s * 128:(s + 1) * 128], id_sb)
                nc.vector.tensor_copy(pT[:, s, :], ps_t2)

            # ---- v-projection: o_full[c, j] = sum_e pooled[c, e] * v_w[j, e] ----
            ps_o = psc.tile([COLS, D], f32, tag="s")
            for s in range(ES):
                nc.tensor.matmul(ps_o, lhsT=pT[:, s, :], rhs=vT_sb[:, s, :],
                                 start=(s == 0), stop=(s == ES - 1))

            # ---- diag extract: oS[g, h*64:(h+1)*64] = ps_o[h*16+g, h*64:(h+1)*64]
            o_sb = small.tile([COLS, D], bf16)
            nc.vector.tensor_copy(o_sb, ps_o)
            oS = small.tile([SLOTS, D], bf16)
            for h in range(H):
                nc.sync.dma_start(
                    out=oS[:, h * DH:(h + 1) * DH],
                    in_=o_sb[h * SLOTS:(h + 1) * SLOTS, h * DH:(h + 1) * DH],
                )

            # ---- oT via PE transposes: [16, 512] -> 4 x [128, 16] ----
            oT = small.tile([128, 4, SLOTS], bf16)
            for s in range(4):
                ps_t3 = pst.tile([128, 128], bf16, tag="tr")
                nc.tensor.transpose(ps_t3[:, 0:SLOTS], oS[:, s * 128:(s + 1) * 128],
                                    id_sb[0:SLOTS, 0:SLOTS])
                nc.vector.tensor_copy(oT[:, s, :], ps_t3[:, 0:SLOTS])

            # ---- out-projection: out[g, j] = sum_i o[g, i] * out_w[j, i] ----
            ps_f = psc.tile([SLOTS, D], f32, tag="s")
            for s in range(4):
                nc.tensor.matmul(ps_f, lhsT=oT[:, s, :], rhs=owT_sb[:, s, :],
                                 start=(s == 0), stop=(s == 3))

            res = small.tile([SLOTS, D], f32)
            nc.vector.tensor_add(res, ps_f, ob_sb)
            nc.sync.dma_start(out=out_d[:, :], in_=res)

    nc.finalize()
    return nc


def _host_prep(graph_emb, qry, q_w, k_w, v_w, in_b, out_w, out_b, ptr, batch):
    graph_emb = np.asarray(graph_emb, dtype=np.float32)
    qry = np.asarray(qry, dtype=np.float32)
    q_w = np.asarray(q_w, dtype=np.float32)
    k_w = np.asarray(k_w, dtype=np.float32)
    v_w = np.asarray(v_w, dtype=np.float32)
    in_b = np.asarray(in_b, dtype=np.float32)
    out_w = np.asarray(out_w, dtype=np.float32)
    out_b = np.asarray(out_b, dtype=np.float32)
    ptr = np.asarray(ptr).astype(np.int64)
    batch = np.asarray(batch).astype(np.int64)

    N = graph_emb.shape[0]
    B = len(ptr) - 1
    n_nodes = ptr[1:] - ptr[:-1]
    max_node = int(n_nodes.max()) + 1
    bs = int(batch.max()) + 1

    # --- mirror the reference's scatter semantics (jnp .at[] wraps negatives,
    # drops OOB, last write wins; valid mask is by slot index) ---
    pos = np.arange(N) - ptr[batch]
    m = np.where(pos < 0, pos + max_node, pos)
    part = (m >= 0) & (m < max_node) & (m < n_nodes[batch])
    idx = np.nonzero(part)[0]
    key = batch[idx] * max_node + m[idx]
    _, first_rev = np.unique(key[::-1], return_index=True)
    keep = idx[::-1][first_rev]
    keep.sort()
    kb = batch[keep]
    counts = np.bincount(kb, minlength=B)
    phantom = n_nodes.astype(np.float64) - counts  # valid-but-unfilled slots

    # --- q-side constant folding (qry is a model parameter) ---
    bq, bk, bv = in_b[:D], in_b[D:2 * D], in_b[2 * D:]
    scale = DH ** -0.5
    q = ((qry.reshape(-1)[-D:] @ q_w.T) + bq) * scale
    qh = q.reshape(H, DH)
    qk = np.stack([qh[h] @ k_w[h * DH:(h + 1) * DH, :] for h in range(H)])  # [8, E]
    qb = np.einsum("hd,hd->h", qh, bk.reshape(H, DH))                        # [8]
    ob_eff = out_b + out_w @ bv

    # --- balanced assignment: 16 graphs per core, boustrophedon by size ---
    order = np.argsort(-counts, kind="stable")
    slot_of = np.empty(B, dtype=np.int64)   # graph -> core*16+slot
    for r in range(SLOTS):
        row = order[r * NCORES:(r + 1) * NCORES]
        seq = range(NCORES) if r % 2 == 0 else range(NCORES - 1, -1, -1)
        for c, gi in zip(seq, row):
            slot_of[gi] = c * SLOTS + r

    nodes_of = [[] for _ in range(B)]
    for n in keep:
        nodes_of[batch[n]].append(n)

    core_loads = np.zeros(NCORES, dtype=np.int64)
    for gi in range(B):
        core_loads[slot_of[gi] // SLOTS] += counts[gi]
    nc_pad = max(512, int(np.ceil(core_loads.max() / 512.0)) * 512)

    exp_qb = np.exp(qb)

    in_maps = []
    for c in range(NCORES):
        rows = []
        ind16 = np.zeros((SLOTS, nc_pad), dtype=BF16)
        ph_col = np.zeros((COLS, 1), dtype=np.float32)
        off = 0
        for s in range(SLOTS):
            gis = np.nonzero(slot_of == c * SLOTS + s)[0]
            if len(gis) == 0:
                continue
            gi = int(gis[0])
            ns = nodes_of[gi]
            rows.extend(ns)
            ind16[s, off:off + len(ns)] = 1
            off += len(ns)
            for h in range(H):
                ph_col[h * SLOTS + s, 0] = phantom[gi]
        emb_c = np.zeros((nc_pad, E), dtype=BF16)
        if rows:
            emb_c[:len(rows)] = graph_emb[np.asarray(rows)].astype(BF16)
        in_maps.append({
            "emb": emb_c,
            "embT": np.ascontiguousarray(
                emb_c.reshape(nc_pad // 512, 512, ES, 128)
                .transpose(0, 3, 2, 1)
                .reshape(nc_pad // 512 * 128, ES * 512)),
            "indT": np.ascontiguousarray(np.tile(ind16, (H, 1))),
            "qk": np.ascontiguousarray(np.repeat(qk, SLOTS, axis=0).T.astype(BF16)),
            "ph": ph_col,
            "vT": np.ascontiguousarray(v_w.T).astype(BF16),
            "owT": np.ascontiguousarray(out_w.T).astype(BF16),
            "ob": np.broadcast_to(ob_eff, (SLOTS, D)).astype(np.float32).copy(),
            "ident": np.eye(128, dtype=BF16),
            "ones": np.ones((128, 1), dtype=BF16),
        })

    meta = {
        "bs": bs,
        "slot_of": slot_of,
        "n_nodes": n_nodes,
        "nc_pad": nc_pad,
    }
    return in_maps, meta


def _assemble(results, meta):
    bs = meta["bs"]
    slot_of = meta["slot_of"]
    n_nodes = meta["n_nodes"]
    out = np.empty((bs, D), dtype=np.float32)
    for b in range(bs):
        sl = int(slot_of[b])
        out[b] = results[sl // SLOTS]["out"][sl % SLOTS]
        if n_nodes[b] <= 0:
            out[b] = np.nan
    return out


def kernel(graph_emb, qry, q_w, k_w, v_w, in_b, out_w, out_b, ptr, batch):
    from concourse.bass_utils import run_bass_kernel_spmd

    in_maps, meta = _host_prep(graph_emb, qry, q_w, k_w, v_w, in_b, out_w,
                               out_b, ptr, batch)
    nc_pad = meta["nc_pad"]
    if nc_pad not in _prog_cache:
        _prog_cache[nc_pad] = _build_program(nc_pad)
    nc = _prog_cache[nc_pad]
    res = run_bass_kernel_spmd(nc, in_maps, list(range(NCORES)))
    return _assemble(res.results, meta)


# revision 11
# speedup vs baseline: 1.4911x; 1.3437x over previous
"""AttentionPooling (ragged segment attention) on 8 Trainium2 NeuronCores.

Full inputs in, full output out. Strategy (data-parallel over graphs):
  - 128 graphs are load-balanced 16-per-core across 8 cores; each core gets
    its graphs' node embeddings (zero-padded to a multiple of 512 rows).
  - The single shared query is a model parameter, so the q-side is constant-
    folded on the host:  qk[h,e] = sum_d q_scaled[h,d]*k_w[h*64+d,e].
  - On device (per core), with cols c = h*16 + s (8 heads x 16 graph slots):
      scoresT[c, n] = sum_e qk_cols[e,c] * embT[e,n]          (PE, bf16)
      e[c, n]       = exp(scoresT + qb[c]) * indicator[c, n]  (ACT + DVE)
      e_cols        = PE-transpose(e)                          [n, c]
      pooled[c, :]  = sum_n e_cols[n,c] * emb[n,:]            (PE, accum)
      colsum[c]     = sum_n e_cols[n,c]  (+ host phantom correction)
      pooled       /= colsum                                  (DVE)
      o[s-block]    = blockdiag v-proj, then out-proj          (PE)
  - Host gathers the 8x[16,512] results back to [bs, 512].
"""

import numpy as np
import ml_dtypes

BF16 = ml_dtypes.bfloat16
FP8 = ml_dtypes.float8_e4m3
QK_SCALE = 64.0
E = 768
D = 512
H = 8
DH = 64
NCORES = 8
SLOTS = 16          # graphs per core
COLS = 128          # H * SLOTS
ES = E // 128       # 6 E-slices of 128

_prog_cache = {}


def _build_program(nc_pad):
    import concourse.bacc as bacc
    import concourse.tile as tile
    import concourse.mybir as mybir

    f32 = mybir.dt.float32
    bf16 = mybir.dt.bfloat16
    f8 = mybir.dt.float8e4
    AF = mybir.ActivationFunctionType

    nc = bacc.Bacc(None, target_bir_lowering=False)

    emb_d = nc.declare_dram_parameter("emb", [nc_pad, E], bf16, isOutput=False)
    NGRP_ = nc_pad // 512
    embT_d = nc.declare_dram_parameter("embT", [NGRP_ * 128, ES * 512], f8, isOutput=False)
    ind_d = nc.declare_dram_parameter("indT", [COLS, nc_pad], bf16, isOutput=False)
    qk_d = nc.declare_dram_parameter("qk", [E, COLS], f8, isOutput=False)
    ph_d = nc.declare_dram_parameter("ph", [COLS, 1], f32, isOutput=False)
    vT_d = nc.declare_dram_parameter("vT", [E, D], bf16, isOutput=False)
    owT_d = nc.declare_dram_parameter("owT", [D, D], bf16, isOutput=False)
    ob_d = nc.declare_dram_parameter("ob", [SLOTS, D], f32, isOutput=False)
    id_d = nc.declare_dram_parameter("ident", [128, 128], bf16, isOutput=False)
    ones_d = nc.declare_dram_parameter("ones", [128, 1], bf16, isOutput=False)
    out_d = nc.declare_dram_parameter("out", [SLOTS, D], f32, isOutput=True)

    NGRP = nc_pad // 512         # 512-node groups
    NCH = nc_pad // 128          # 128-node chunks

    with tile.TileContext(nc) as tc:
        with (
            tc.tile_pool(name="const", bufs=1) as const,
            tc.tile_pool(name="embT_p", bufs=3) as embT_p,
            tc.tile_pool(name="emb_p", bufs=10) as emb_p,
            tc.tile_pool(name="e_p", bufs=3) as e_p,
            tc.tile_pool(name="ec_p", bufs=6) as ec_p,
            tc.tile_pool(name="small", bufs=1) as small,
            tc.tile_pool(name="psc", bufs=2, space="PSUM") as psc,
            tc.tile_pool(name="pst", bufs=3, space="PSUM") as pst,
            tc.tile_pool(name="pacc", bufs=1, space="PSUM") as pacc,
        ):
            # ---- critical-path constants first: qk, then group-0 embT ----
            qk_sb = const.tile([128, ES, COLS], f8)
            nc.sync.dma_start(out=qk_sb, in_=qk_d.rearrange("(s p) c -> p s c", p=128))

            # ---- persistent accumulators (PSUM) ----
            ps_pool = pacc.tile([COLS, E], f32)      # pooled_u, 2 banks
            ps_cs = pacc.tile([COLS, 1], f32)        # col sums, 1 bank

            # embT is shipped pre-grouped: [NGRP, 128, ES, 512] with each
            # group's block contiguous in DRAM for clean big DMAs.
            embT_r = embT_d.rearrange("(g p) (s n) -> g p s n", p=128, n=512)

            def load_et(g):
                et = embT_p.tile([128, ES, 512], f8, tag="et")
                nc.sync.dma_start(out=et, in_=embT_r[g])
                return et

            def emit_scores(g, et):
                ps_s = psc.tile([COLS, 512], f32, tag="s")
                for s in range(ES):
                    nc.tensor.matmul(
                        ps_s, lhsT=qk_sb[:, s, :], rhs=et[:, s, :],
                        start=(s == 0), stop=(s == ES - 1),
                    )
                # e = exp(scores); mask by indicator (qb folded out, see host)
                e_sb = e_p.tile([COLS, 512], bf16, tag="e")
                nc.scalar.activation(out=e_sb, in_=ps_s, func=AF.Exp, scale=1.0 / QK_SCALE)
                em_sb = e_p.tile([COLS, 512], bf16, tag="em")
                nc.vector.tensor_mul(em_sb, e_sb, ind_sb[:, g * 512:(g + 1) * 512])
                embts = []
                for j in range(4):
                    ch = g * 4 + j
                    embt = emb_p.tile([128, E], bf16)
                    nc.sync.dma_start(out=embt, in_=emb_d[ch * 128:(ch + 1) * 128, :])
                    embts.append(embt)
                return em_sb, embts

            def emit_pool(g, em_sb, embts):
                for j in range(4):
                    ch = g * 4 + j
                    # e_cols chunk: [128 nodes, 128 cols] via PE transpose
                    ps_t = pst.tile([128, 128], bf16, tag="tr")
                    nc.tensor.transpose(ps_t, em_sb[:, j * 128:(j + 1) * 128], id_sb)
                    ec = ec_p.tile([128, COLS], bf16)
                    nc.vector.tensor_copy(ec, ps_t)

                    embt = embts[j]
                    st = (ch == 0)
                    sp = (ch == NCH - 1)
                    nc.tensor.matmul(ps_pool[:, 0:512], lhsT=ec, rhs=embt[:, 0:512],
                                     start=st, stop=sp)
                    nc.tensor.matmul(ps_pool[:, 512:768], lhsT=ec, rhs=embt[:, 512:768],
                                     start=st, stop=sp)
                    nc.tensor.matmul(ps_cs, lhsT=ec, rhs=ones_sb, start=st, stop=sp)

            # Software pipeline: PE stays busy on group g+1's scores while
            # ACT/DVE produce group g's masked-exp, whose transposes+pools
            # are emitted (in PE program order) after those scores.
            ets = {0: load_et(0)}

            # secondary constants (needed a bit later than qk/et0)
            id_sb = const.tile([128, 128], bf16)
            nc.sync.dma_start(out=id_sb, in_=id_d[:, :])
            ind_sb = const.tile([COLS, nc_pad], bf16)
            nc.sync.dma_start(out=ind_sb, in_=ind_d[:, :])
            ones_sb = const.tile([128, 1], bf16)
            nc.sync.dma_start(out=ones_sb, in_=ones_d[:, :])

            prev = None
            for g in range(NGRP):
                if g + 1 < NGRP:
                    ets[g + 1] = load_et(g + 1)
                em, embts = emit_scores(g, ets.pop(g))
                if prev is not None:
                    emit_pool(*prev)
                prev = (g, em, embts)
            emit_pool(*prev)

            # tail-only constants
            ph_sb = const.tile([COLS, 1], f32)
            nc.sync.dma_start(out=ph_sb, in_=ph_d[:, :])
            vT_sb = const.tile([128, ES, D], bf16)
            nc.sync.dma_start(out=vT_sb, in_=vT_d.rearrange("(s p) c -> p s c", p=128))
            owT_sb = const.tile([128, 4, D], bf16)
            nc.sync.dma_start(out=owT_sb, in_=owT_d.rearrange("(s p) c -> p s c", p=128))
            ob_sb = const.tile([SLOTS, D], f32)
            nc.sync.dma_start(out=ob_sb, in_=ob_d[:, :])

            # ---- normalize ----
            cs_sb = small.tile([COLS, 1], f32)
            nc.vector.tensor_add(cs_sb, ps_cs, ph_sb)
            rec_sb = small.tile([COLS, 1], f32)
            nc.vector.reciprocal(rec_sb, cs_sb)
            pooled = small.tile([COLS, E], bf16)
            nc.vector.tensor_scalar_mul(pooled, in0=ps_pool, scalar1=rec_sb)

            # ---- pooledT via PE transposes (phase-separated) ----
            pT = small.tile([128, ES, COLS], bf16)
            t2s = []
            for s in range(ES):
                ps_t2 = pst.tile([128, 128], bf16, tag="tr")
                nc.tensor.transpose(ps_t2, pooled[:, s * 128:(s + 1) * 128], id_sb)
                t2s.append(ps_t2)
            for s in range(ES):
                nc.vector.tensor_copy(pT[:, s, :], t2s[s])

            # ---- v-projection: o_full[c, j] = sum_e pooled[c, e] * v_w[j, e] ----
            ps_o = psc.tile([COLS, D], f32, tag="s")
            for s in range(ES):
                nc.tensor.matmul(ps_o, lhsT=pT[:, s, :], rhs=vT_sb[:, s, :],
                                 start=(s == 0), stop=(s == ES - 1))

            # ---- diag extract: oS[g, h*64:(h+1)*64] = ps_o[h*16+g, h*64:(h+1)*64]
            o_sb = small.tile([COLS, D], bf16)
            nc.vector.tensor_copy(o_sb, ps_o)
            oS = small.tile([SLOTS, D], bf16)
            for h in range(H):
                nc.sync.dma_start(
                    out=oS[:, h * DH:(h + 1) * DH],
                    in_=o_sb[h * SLOTS:(h + 1) * SLOTS, h * DH:(h + 1) * DH],
                )

            # ---- oT via PE transposes: [16, 512] -> 4 x [128, 16] ----
            oT = small.tile([128, 4, SLOTS], bf16)
            t3s = []
            for s in range(4):
                ps_t3 = pst.tile([128, 128], bf16, tag="tr")
                nc.tensor.transpose(ps_t3[:, 0:SLOTS], oS[:, s * 128:(s + 1) * 128],
                                    id_sb[0:SLOTS, 0:SLOTS])
                t3s.append(ps_t3)
            for s in range(4):
                nc.vector.tensor_copy(oT[:, s, :], t3s[s][:, 0:SLOTS])

            # ---- out-projection: out[g, j] = sum_i o[g, i] * out_w[j, i] ----
            ps_f = psc.tile([SLOTS, D], f32, tag="s")
            for s in range(4):
                nc.tensor.matmul(ps_f, lhsT=oT[:, s, :], rhs=owT_sb[:, s, :],
                                 start=(s == 0), stop=(s == 3))

            res = small.tile([SLOTS, D], f32)
            nc.vector.tensor_add(res, ps_f, ob_sb)
            nc.sync.dma_start(out=out_d[:, :], in_=res)

    nc.finalize()
    return nc


def _host_prep(graph_emb, qry, q_w, k_w, v_w, in_b, out_w, out_b, ptr, batch):
    graph_emb = np.asarray(graph_emb, dtype=np.float32)
    qry = np.asarray(qry, dtype=np.float32)
    q_w = np.asarray(q_w, dtype=np.float32)
    k_w = np.asarray(k_w, dtype=np.float32)
    v_w = np.asarray(v_w, dtype=np.float32)
    in_b = np.asarray(in_b, dtype=np.float32)
    out_w = np.asarray(out_w, dtype=np.float32)
    out_b = np.asarray(out_b, dtype=np.float32)
    ptr = np.asarray(ptr).astype(np.int64)
    batch = np.asarray(batch).astype(np.int64)

    N = graph_emb.shape[0]
    B = len(ptr) - 1
    n_nodes = ptr[1:] - ptr[:-1]
    max_node = int(n_nodes.max()) + 1
    bs = int(batch.max()) + 1

    # --- mirror the reference's scatter semantics (jnp .at[] wraps negatives,
    # drops OOB, last write wins; valid mask is by slot index) ---
    pos = np.arange(N) - ptr[batch]
    m = np.where(pos < 0, pos + max_node, pos)
    part = (m >= 0) & (m < max_node) & (m < n_nodes[batch])
    idx = np.nonzero(part)[0]
    key = batch[idx] * max_node + m[idx]
    _, first_rev = np.unique(key[::-1], return_index=True)
    keep = idx[::-1][first_rev]
    keep.sort()
    kb = batch[keep]
    counts = np.bincount(kb, minlength=B)
    phantom = n_nodes.astype(np.float64) - counts  # valid-but-unfilled slots

    # --- q-side constant folding (qry is a model parameter) ---
    bq, bk, bv = in_b[:D], in_b[D:2 * D], in_b[2 * D:]
    scale = DH ** -0.5
    q = ((qry.reshape(-1)[-D:] @ q_w.T) + bq) * scale
    qh = q.reshape(H, DH)
    qk = np.stack([qh[h] @ k_w[h * DH:(h + 1) * DH, :] for h in range(H)])  # [8, E]
    qb = np.einsum("hd,hd->h", qh, bk.reshape(H, DH))                        # [8]
    ob_eff = out_b + out_w @ bv

    # --- balanced assignment: 16 graphs per core, boustrophedon by size ---
    order = np.argsort(-counts, kind="stable")
    slot_of = np.empty(B, dtype=np.int64)   # graph -> core*16+slot
    for r in range(SLOTS):
        row = order[r * NCORES:(r + 1) * NCORES]
        seq = range(NCORES) if r % 2 == 0 else range(NCORES - 1, -1, -1)
        for c, gi in zip(seq, row):
            slot_of[gi] = c * SLOTS + r

    nodes_of = [[] for _ in range(B)]
    for n in keep:
        nodes_of[batch[n]].append(n)

    core_loads = np.zeros(NCORES, dtype=np.int64)
    for gi in range(B):
        core_loads[slot_of[gi] // SLOTS] += counts[gi]
    nc_pad = max(512, int(np.ceil(core_loads.max() / 512.0)) * 512)

    exp_qb = np.exp(qb)

    in_maps = []
    for c in range(NCORES):
        rows = []
        ind16 = np.zeros((SLOTS, nc_pad), dtype=BF16)
        ph_col = np.zeros((COLS, 1), dtype=np.float32)
        off = 0
        for s in range(SLOTS):
            gis = np.nonzero(slot_of == c * SLOTS + s)[0]
            if len(gis) == 0:
                continue
            gi = int(gis[0])
            ns = nodes_of[gi]
            rows.extend(ns)
            ind16[s, off:off + len(ns)] = 1
            off += len(ns)
            for h in range(H):
                ph_col[h * SLOTS + s, 0] = phantom[gi]
        emb_c = np.zeros((nc_pad, E), dtype=BF16)
        if rows:
            emb_c[:len(rows)] = graph_emb[np.asarray(rows)].astype(BF16)
        in_maps.append({
            "emb": emb_c,
            "embT": np.ascontiguousarray(
                emb_c.reshape(nc_pad // 512, 512, ES, 128)
                .transpose(0, 3, 2, 1)
                .reshape(nc_pad // 512 * 128, ES * 512)).astype(FP8),
            "indT": np.ascontiguousarray(np.tile(ind16, (H, 1))),
            "qk": np.ascontiguousarray((np.repeat(qk, SLOTS, axis=0) * QK_SCALE).T.astype(FP8)),
            "ph": ph_col,
            "vT": np.ascontiguousarray(v_w.T).astype(BF16),
            "owT": np.ascontiguousarray(out_w.T).astype(BF16),
            "ob": np.broadcast_to(ob_eff, (SLOTS, D)).astype(np.float32).copy(),
            "ident": np.eye(128, dtype=BF16),
            "ones": np.ones((128, 1), dtype=BF16),
        })

    meta = {
        "bs": bs,
        "slot_of": slot_of,
        "n_nodes": n_nodes,
        "nc_pad": nc_pad,
    }
    return in_maps, meta


def _assemble(results, meta):
    bs = meta["bs"]
    slot_of = meta["slot_of"]
    n_nodes = meta["n_nodes"]
    out = np.empty((bs, D), dtype=np.float32)
    for b in range(bs):
        sl = int(slot_of[b])
        out[b] = results[sl // SLOTS]["out"][sl % SLOTS]
        if n_nodes[b] <= 0:
            out[b] = np.nan
    return out


def kernel(graph_emb, qry, q_w, k_w, v_w, in_b, out_w, out_b, ptr, batch):
    from concourse.bass_utils import run_bass_kernel_spmd

    in_maps, meta = _host_prep(graph_emb, qry, q_w, k_w, v_w, in_b, out_w,
                               out_b, ptr, batch)
    nc_pad = meta["nc_pad"]
    if nc_pad not in _prog_cache:
        _prog_cache[nc_pad] = _build_program(nc_pad)
    nc = _prog_cache[nc_pad]
    res = run_bass_kernel_spmd(nc, in_maps, list(range(NCORES)))
    return _assemble(res.results, meta)
